# revision 33
# baseline (speedup 1.0000x reference)
"""Trainium2 Bass kernel for a stride-2 4x4 ConvTranspose2d with
per-kernel-position bias (bias added before the overlap-add fold).

Shapes (hardcoded):
  x:      (8, 256, 64, 64)  f32
  weight: (128, 256, 4, 4)  f32
  bias:   (128, 4, 4)       f32
  out:    (8, 128, 130, 130) f32   [nh = (64-1)*2+4 = 130]

Strategy: data-parallel over batch - one sample per NeuronCore, 8 cores.
Per core the deconv is computed as 4 output-phase planes (p%2, q%2), each
a 65x65 image. Each phase plane is the sum of 8 shifted matmuls (the 4
kernel positions sharing that parity x 2 contraction halves) accumulated
directly in PSUM:

  plane[o, P, Q] = sum_{kt, a,b in {0,1}} W[:, kt128:, py+2a, px+2b]^T @ xp[kt, P-a+1, Q-b+1]

All matmul inputs are bf16 (rel err ~4e-3 of output scale, far under the
2e-2 gate); bf16 enables FWL so LDWEIGHTS fully overlaps streaming and
the PE runs at its ~2.4GHz roofline.

DMA schedule (the trace-driven part): the kernel window is dominated by
the PE stream (~58us), so every input byte must arrive just ahead of the
chunk that consumes it.  xp is loaded as 6 row-bands, all on the sync
HWDGE ring (FIFO per ring -> bands drain strictly in row order with no
inter-DMA gap); the first band is tiny (5 rows) so the first real matmul
can start as soon as ~2.5us after the engines enter the kernel body.
Weights go on the scalar HWDGE ring, phase-0 split by contraction half
so the first LDWEIGHTS gates on a 128KB transfer only; the bulk
phase-1..3 weights are completion-chained behind band 4 so they don't
round-robin-steal bandwidth from the bands the early chunks need.
A short warm-up matmul train (dep-free) keeps the PE busy during the
input load so HAM un-throttles to 2.4GHz quickly.  The tail is cut by
draining+storing the last 2-row chunk of phase 3 as its own tiny DMA.

The per-kernel-position bias with all edge/corner corrections baked in
is 3 host-precomputed f32 rows (top/interior/bottom) per phase; each
chunk drain is a single tensor_add (stride-0 broadcast bias AP) from
PSUM into a bf16 plane.  The host interleaves the 4 bf16 planes into the
strided f32 (130,130) output.
"""

import numpy as np

B, CI, H, W = 8, 256, 64, 64
CO, KH, KW = 128, 4, 4
NH = NW = 130
NP = 65          # phase plane side
PADH = 66        # padded x rows (+1 top, +1 bottom)
PADW = 68        # padded x cols (+1 left, +3 right; row stride 2*68 elems)
NCORES = 8

# 10 chunks: steady 7-row chunks with a tiny 2-row tail (small final
# drain+DMA).  Chunks 0 AND 1 both read inside band 0, so the stream's
# first ~3us of work gates on a single DMA-completion semaphore -
# completion semaphores fire 0.9-2us after the data lands (HBM receipt
# under load), and per-chunk gating semaphores proved to be the main
# run-to-run variance source (0.3-1.8us stalls).
CHUNK_ROWS = [7, 7, 7, 7, 7, 7, 7, 7, 7, 2]
assert sum(CHUNK_ROWS) == NP
# xp row-bands, loaded in consumption order on one HWDGE ring; band 0
# covers chunks 0-1, every later chunk's gating semaphore fires >=1.5us
# before its consumer chunk starts.
BANDS = [(0, 15), (15, 22), (22, 29), (29, 36), (36, 43),
         (43, 50), (50, 57), (57, 66)]
# warm-up matmul lengths (dep-free, cover the input-load window at cold
# clock ~N/1.2 ns each; fine 64-col tail to minimize leftover delay).
# Overshoot is cheap (leftover warm-ups delay the stream by <=their own
# length); undershoot idles the PE, which can reset the HAM activity
# window and restart the ~3.4us half-clock ramp.
WARM_NS = [256] * 8 + [64] * 28


def _build_nc():
    import concourse.mybir as mybir
    import concourse.tile as tile
    from concourse import bacc
    from concourse.tile_rust import add_dep_helper

    f32 = mybir.dt.float32
    bf16 = mybir.dt.bfloat16

    nc = bacc.Bacc(None)
    # xp layout: [i=128, row=66, kt=2, col=68] bf16 - kt inner so a row
    # range is one contiguous DMA descriptor.
    xp_ext = nc.declare_dram_parameter("xp", [128, PADH * 2 * PADW], bf16, isOutput=False)
    # wt layout: [i=128, phase=4, kt=2, tap=4, o=128] bf16 - phase outer so
    # phase-0 weights arrive first, kt next so each contraction half is one
    # contiguous slice.
    wt_ext = nc.declare_dram_parameter("wt", [128, 4 * 2 * 4 * CO], bf16, isOutput=False)
    # bias rows: [o=128, phase=4, kind=3 (top/int/bottom), 65] f32
    bv_ext = nc.declare_dram_parameter("bv", [128, 4 * 3 * NP], f32, isOutput=False)
    out_ext = nc.declare_dram_parameter("out", [4, CO, NP * NP], bf16, isOutput=True)

    with tile.TileContext(nc) as tc:
        with (
            tc.tile_pool(name="const", bufs=1) as cpool,
            tc.tile_pool(name="psum", bufs=6, space="PSUM") as ppool,
        ):
            xp_t = cpool.tile([128, PADH, 2, PADW], bf16, tag="xp", name="xp")
            w_t = cpool.tile([128, 4, 2, 4, CO], bf16, tag="w", name="w")
            bv_t = cpool.tile([128, 4, 3, NP], f32, tag="bv", name="bv")
            planes = [
                cpool.tile([128, NP, NP], bf16, tag=f"plane{p}", name=f"plane{p}")
                for p in range(4)
            ]

            # xp bands: all on the sync HWDGE ring.  Same-ring DMAs drain
            # FIFO, so chaining trigger order (sync=False: scheduler-order
            # only, no completion semaphore) keeps the bands streaming
            # back-to-back in row order at full ring bandwidth.
            # The SDMA engines round-robin rings at descriptor-batch
            # granularity roughly proportional to descriptor size, so any
            # concurrent second ring starves the critical prefix (measured
            # ~80GB/s vs 215GB/s, both directions, across rounds).  Hence
            # EVERYTHING the first ~15us of the stream needs goes on the
            # sync ring, serial FIFO, in consumption order:
            #   [w-phase0, band0, band1, band2, bias, band3, band4, band5]
            # (bias is only needed by the PSUM drains, which have ~6 chunks
            # of slack behind the matmuls.)  Each DMA trigger instruction
            # costs ~600ns on the issuing engine and descriptors only exist
            # after the trigger, so the prefix uses few, consumption-ordered
            # transfers rather than many small ones.
            band_dmas = []
            prev = None
            for bi, (r0, r1) in enumerate(BANDS):
                dmah = nc.sync.dma_start(
                    xp_t[:, r0:r1], xp_ext[:, r0 * 2 * PADW: r1 * 2 * PADW]
                )
                if bi == 0:
                    # w-phase0-kt0 rides ahead of the first rows
                    wk0 = nc.sync.dma_start(w_t[:, 0, 0], wt_ext[:, 0:4 * CO])
                    add_dep_helper(dmah.ins, wk0.ins, sync=False, reason="sync ring order")
                else:
                    add_dep_helper(dmah.ins, prev.ins, sync=False, reason="sync ring order")
                prev = dmah
                band_dmas.append(dmah)
                if bi == 0:
                    wk1 = nc.sync.dma_start(w_t[:, 0, 1], wt_ext[:, 4 * CO:2 * 4 * CO])
                    add_dep_helper(wk1.ins, prev.ins, sync=False, reason="sync ring order")
                    prev = wk1
                elif bi == 2:
                    bv = nc.sync.dma_start(bv_t[:], bv_ext[:])
                    add_dep_helper(bv.ins, prev.ins, sync=False, reason="sync ring order")
                    prev = bv

            # The 768KB phase-1..3 weight bulk rides the scalar ring,
            # completion-chained behind band 4: phase-1 weights aren't
            # needed until ~15us after the stream starts, and starting the
            # bulk early would halve the bandwidth the early bands get.
            w123 = nc.scalar.dma_start(w_t[:, 1:4], wt_ext[:, 2 * 4 * CO:])
            add_dep_helper(w123.ins, band_dmas[3].ins, sync=True,
                           reason="bulk weights wait until early bands done")

            # PE warm-up: dummy bf16 matmuls with no input deps run during
            # the input-load window so HAM un-throttles (needs ~3.4us of
            # sustained PE activity) by the time real matmuls begin.
            warm_in = cpool.tile([128, 256], bf16, tag="warm_in", name="warm_in")
            warm_ps = ppool.tile([128, 256], f32, tag="warm_ps", name="warm_ps", bufs=1)
            nc.vector.memset(warm_in[:], 1.0)
            for n in WARM_NS:
                nc.tensor.matmul(
                    warm_ps[:, 0:n], warm_in[:, 0:128], warm_in[:, 0:n],
                    start=True, stop=True,
                )

            starts = [sum(CHUNK_ROWS[:i]) for i in range(len(CHUNK_ROWS))]
            first_out = {"sync": None, "scalar": None}
            for pidx in range(4):
                for ci, pn in enumerate(CHUNK_ROWS):
                    pstart = starts[ci]
                    nn = pn * NP
                    ps = ppool.tile([128, 512], f32, tag="acc", name=f"acc{pidx}_{ci}")
                    # kt-major so the first 4 matmuls of chunk 0 only gate
                    # on the first weight half (wk0).
                    mm = 0
                    for kt in (0, 1):
                        for a in (0, 1):
                            for b in (0, 1):
                                lhsT = w_t[:, pidx, kt, a * 2 + b, :]
                                rhs = xp_t[
                                    :,
                                    pstart + 1 - a: pstart + 1 - a + pn,
                                    kt,
                                    1 - b: 1 - b + NP,
                                ]
                                nc.tensor.matmul(
                                    ps[:, 0:nn], lhsT, rhs,
                                    start=(mm == 0), stop=(mm == 7),
                                )
                                mm += 1
                    # drain: PSUM + bias rows -> bf16 plane.
                    # bias row kinds: 0=top (P=0), 1=interior, 2=bottom (P=64)
                    pl = planes[pidx]
                    if ci == 0:
                        nc.vector.tensor_add(
                            pl[:, 0:1, :],
                            ps[:, 0:NP].rearrange("p (a b) -> p a b", b=NP),
                            bv_t[:, pidx, 0:1, :],
                        )
                        nc.vector.tensor_add(
                            pl[:, 1:7, :],
                            ps[:, NP:7 * NP].rearrange("p (a b) -> p a b", b=NP),
                            bv_t[:, pidx, 1:2, :].broadcast_to([128, 6, NP]),
                        )
                    elif ci == 9:
                        # rows 63 (interior) + 64 (bottom) = bias kinds 1,2
                        # which are contiguous in bv - single fused drain so
                        # the kernel-tail store gates on one vector op.
                        nc.vector.tensor_add(
                            pl[:, 63:65, :],
                            ps[:, 0:2 * NP].rearrange("p (a b) -> p a b", b=NP),
                            bv_t[:, pidx, 1:3, :],
                        )
                    else:
                        nc.vector.tensor_add(
                            pl[:, pstart:pstart + pn, :],
                            ps[:, 0:nn].rearrange("p (a b) -> p a b", b=NP),
                            bv_t[:, pidx, 1:2, :].broadcast_to([128, pn, NP]),
                        )
                    # output DMAs: phases 0-1 on the scalar ring (idle after
                    # the weight loads), 2-3 on the sync ring (idle after
                    # the bands).  The ci==9 store always goes on scalar so
                    # its ~600ns trigger instruction never serializes ahead
                    # of the final tiny store on the sync engine; the last
                    # store of phase 3 covers only 2 rows so the kernel
                    # tail is one drain + a 33KB DMA.
                    eng, ename = (nc.scalar, "scalar") if pidx < 2 else (nc.sync, "sync")
                    o = None
                    if ci == 4:
                        o = eng.dma_start(out_ext[pidx, :, 0:35 * NP], pl[:, 0:35, :])
                    elif ci == 7:
                        # phase 3's mid store rides scalar so the sync ring
                        # is drained well before the final tiny store (HBM
                        # receipt latency grows when the ring is busy)
                        e7 = nc.scalar if pidx == 3 else eng
                        if pidx == 3:
                            ename = "scalar"
                        o = e7.dma_start(out_ext[pidx, :, 35 * NP:56 * NP], pl[:, 35:56, :])
                    elif ci == 8:
                        o = nc.scalar.dma_start(out_ext[pidx, :, 56 * NP:63 * NP], pl[:, 56:63, :])
                        ename = "scalar"
                    elif ci == 9:
                        o = eng.dma_start(out_ext[pidx, :, 63 * NP:], pl[:, 63:65, :])
                    if o is not None and first_out[ename] is None:
                        first_out[ename] = o
                        tail = w123 if ename == "scalar" else band_dmas[-1]
                        add_dep_helper(o.ins, tail.ins, sync=False,
                                       reason="outs trigger after input loads")
    nc.compile()
    return nc


def _host_prep(x, weight, bias):
    import ml_dtypes

    # padded, i-tiled x: [B, 128, row=66, kt=2, col=68] bf16
    xp = np.zeros((B, 128, PADH, 2, PADW), dtype=ml_dtypes.bfloat16)
    xr = x.reshape(B, 2, 128, H, W).transpose(0, 2, 3, 1, 4)  # b,i,h,kt,w
    xp[:, :, 1:65, :, 1:65] = xr.astype(ml_dtypes.bfloat16)
    xp = np.ascontiguousarray(xp.reshape(B, 128, PADH * 2 * PADW))

    # weights as lhsT: wt[i, phase, kt, tap=(a,b), o] = weight[o, kt*128+i, py+2a, px+2b]
    wr = weight.reshape(CO, 2, 128, 4, 4)
    wt = np.empty((128, 4, 2, 4, CO), dtype=ml_dtypes.bfloat16)
    for py in range(2):
        for px in range(2):
            for a in range(2):
                for bb in range(2):
                    wt[:, py * 2 + px, :, a * 2 + bb, :] = (
                        wr[:, :, :, py + 2 * a, px + 2 * bb]
                        .transpose(2, 1, 0).astype(ml_dtypes.bfloat16)
                    )
    wt = np.ascontiguousarray(wt).reshape(128, 4 * 2 * 4 * CO)

    # bias rows [o, phase, kind=3, 65] f32: kind 0 = P=0 (top), 1 = interior
    # P, 2 = P=64 (bottom); per-column validity baked in.
    bv = np.zeros((128, 4, 3, NP), dtype=np.float32)
    bias = bias.astype(np.float32)
    for py in range(2):
        for px in range(2):
            p = py * 2 + px
            for kind, avalid in ((0, (0,)), (1, (0, 1)), (2, (1,))):
                for q in range(NP):
                    s = np.zeros(128, dtype=np.float32)
                    for a in range(2):
                        if a not in avalid:
                            continue
                        for b2 in range(2):
                            if q == 0 and b2 == 1:
                                continue
                            if q == NP - 1 and b2 == 0:
                                continue
                            s += bias[:, py + 2 * a, px + 2 * b2]
                    bv[:, p, kind, q] = s
    bv = np.ascontiguousarray(bv.reshape(128, 4 * 3 * NP))
    return xp, wt, bv


_NC_CACHE = {}


def _get_nc():
    if "nc" not in _NC_CACHE:
        _NC_CACHE["nc"] = _build_nc()
    return _NC_CACHE["nc"]


def kernel(x, weight, bias, _trace=False, _trace_kwargs=None):
    from concourse.bass_utils import run_bass_kernel_spmd

    x = np.asarray(x, dtype=np.float32)
    weight = np.asarray(weight, dtype=np.float32)
    bias = np.asarray(bias, dtype=np.float32)
    xp, wt, bv = _host_prep(x, weight, bias)

    nc = _get_nc()
    in_maps = [{"xp": xp[b], "wt": wt, "bv": bv} for b in range(B)]
    res = run_bass_kernel_spmd(
        nc, in_maps, list(range(NCORES)),
        trace=_trace, **(_trace_kwargs or {}),
    )
    out = np.empty((B, CO, NH, NW), dtype=np.float32)
    for b in range(B):
        ph = res.results[b]["out"].reshape(4, CO, NP, NP).astype(np.float32)
        for py in range(2):
            for px in range(2):
                out[b, :, py::2, px::2] = ph[py * 2 + px]
    if _trace:
        kernel._last_results = res
    return out
